# revision 17
# baseline (speedup 1.0000x reference)
"""Trainium2 Bass kernel for a GRU attention-decoder step, 8-way sharded.

Sharding:
  - Attention + context: data-parallel over batch (8 rows/core), with the
    algebraic collapse scores = enc . (attn_W[:,H:]^T v)  (the last_hidden
    part of the Bahdanau energy is constant over t and cancels in softmax).
  - Embedding: vocab-sharded table, masked indirect-DMA gather + AllReduce.
  - GRU: hidden-sharded (128 dims/core), AllGather of context rows and
    h_new^T.
  - Vocab projection: out_W row-sharded (6250 rows/core), bf16 weight
    stream + f32 psum accumulate, log-softmax via AllGather of per-core
    (max, sumexp) with per-section partial sums overlapped with the stream.

Self-contained: hardcodes all shapes; builds+compiles the Bass program once
per process and runs it on cores 0-7 via run_bass_kernel_spmd.
"""
import os
import numpy as np
import ml_dtypes

import concourse.bass as bass
import concourse.tile as tile
from concourse import bacc, mybir
from concourse.bass_utils import run_bass_kernel_spmd
from concourse.masks import make_identity

V, IN, H, B, T = 50000, 512, 1024, 64, 256
NCORES = 8
VS = V // NCORES          # 6250 vocab rows per core
BS = B // NCORES          # 8 batch rows per core
HS = H // NCORES          # 128 hidden dims per core
K2H = 2 * H               # 2048 contraction dim of vocab matmul
KX = IN + H               # 1536 GRU input width

f32 = mybir.dt.float32
f32r = mybir.dt.float32r
bf16 = mybir.dt.bfloat16
i32 = mybir.dt.int32
AF = mybir.ActivationFunctionType
ALU = mybir.AluOpType
X = mybir.AxisListType.X

SECS = [2048, 2048, 2048, 106]
SEC_OFF = [0, 2048, 4096, 6144]

_CACHED = {}


def _build():
    nc = bacc.Bacc("TRN2", target_bir_lowering=False, debug=False,
                   num_devices=NCORES)
    rg = [list(range(NCORES))]

    # ---- per-core external inputs ----
    encT = nc.dram_tensor("encT", [H, BS * T], bf16, kind="ExternalInput")
    enc2 = nc.dram_tensor("enc2", [T, BS * H], bf16, kind="ExternalInput")
    w2 = nc.dram_tensor("w2", [128, 8], bf16, kind="ExternalInput")
    emb_tab = nc.dram_tensor("emb_tab", [VS, IN], f32, kind="ExternalInput")
    idx = nc.dram_tensor("idx", [B, 1], i32, kind="ExternalInput")
    lhT = nc.dram_tensor("lhT", [128, 8 * B], f32r, kind="ExternalInput")
    lh_sl = nc.dram_tensor("lh_sl", [HS, B], f32, kind="ExternalInput")
    wihT = nc.dram_tensor("wihT", [128, 12 * 3 * HS], f32r, kind="ExternalInput")
    whhT = nc.dram_tensor("whhT", [128, 8 * 3 * HS], f32r, kind="ExternalInput")
    gbias = nc.dram_tensor("gbias", [HS, 4], f32, kind="ExternalInput")
    owT = nc.dram_tensor("owT", [K2H + 1, VS], bf16, kind="ExternalInput")

    logp_o = nc.dram_tensor("logp", [B, VS], f32, kind="ExternalOutput")
    hnT_o = nc.dram_tensor("hnT", [H, B], f32, kind="ExternalOutput")

    with tile.TileContext(nc) as tc:
        with (
            tc.tile_pool(name="const", bufs=1) as cpool,
            tc.tile_pool(name="enc", bufs=3) as encp,
            tc.tile_pool(name="enc2", bufs=4) as enc2p,
            tc.tile_pool(name="small", bufs=1) as sp,
            tc.tile_pool(name="wstream", bufs=12) as wp,
            tc.tile_pool(name="brow", bufs=1) as bp,
            tc.tile_pool(name="scratch", bufs=1) as scr,
            tc.tile_pool(name="logits", bufs=1) as lp,
            tc.tile_pool(name="psA", bufs=4, space="PSUM") as psA,
            tc.tile_pool(name="psB", bufs=4, space="PSUM") as psB,
            tc.tile_pool(name="dram", bufs=1, space="DRAM") as dr,
        ):
            # ================= setup =================
            ident = cpool.tile([128, 128], f32)
            make_identity(nc, ident[:])
            ones_f = cpool.tile([1, 128], f32)
            nc.vector.memset(ones_f[:], 1.0)
            ones = cpool.tile([1, 128], f32r)
            nc.vector.tensor_copy(ones[:], ones_f[:])
            ones_b = cpool.tile([1, B], bf16)
            nc.vector.tensor_copy(ones_b[:], ones_f[:1, :B])

            # embedding gather + AllReduce first: tiny DMAs, early doorbell
            idx_sb = cpool.tile([B, 1], i32)
            nc.gpsimd.dma_start(idx_sb[:], idx[:])
            embp = sp.tile([B, IN], f32)
            nc.vector.memset(embp[:], 0.0)
            nc.gpsimd.indirect_dma_start(
                out=embp[:], out_offset=None,
                in_=emb_tab[:],
                in_offset=bass.IndirectOffsetOnAxis(ap=idx_sb[:, :1], axis=0),
                bounds_check=VS - 1, oob_is_err=False)
            ar_in = dr.tile([B, IN], f32)
            nc.gpsimd.dma_start(ar_in[:], embp[:])
            ar_out = dr.tile([B, IN], f32, addr_space="Shared")
            nc.gpsimd.collective_compute(
                "AllReduce", ALU.add, replica_groups=rg,
                ins=[ar_in.opt()], outs=[ar_out.opt()])

            # dense setup DMAs (host pre-swizzled to [128, free] layouts)
            w2_sb = cpool.tile([128, 8], bf16)
            nc.gpsimd.dma_start(w2_sb[:], w2.ap())
            lhT_sb = cpool.tile([128, 8 * B], f32r)
            nc.gpsimd.dma_start(lhT_sb[:], lhT.ap())
            lhsl_sb = cpool.tile([HS, B], f32)
            nc.gpsimd.dma_start(lhsl_sb[:], lh_sl[:])
            wih_sb = cpool.tile([128, 12 * 3 * HS], f32r)
            nc.gpsimd.dma_start(wih_sb[:], wihT.ap())
            whh_sb = cpool.tile([128, 8 * 3 * HS], f32r)
            nc.gpsimd.dma_start(whh_sb[:], whhT.ap())
            gb_sb = cpool.tile([HS, 4], f32)
            nc.gpsimd.dma_start(gb_sb[:], gbias[:])
            xt_sb = sp.tile([128, 12 * B], f32r)

            # ================= attention =================
            # pass 1: scores[b,t] = sum_h enc[h,b,t] * w2[h]
            sc_ps = []
            for q in range(4):
                p = psA.tile([128, 512], f32, space="PSUM", tag="ps",
                             name=f"sc_{q}")
                sc_ps.append(p)
            for k in range(8):
                ek = encp.tile([128, 2048], bf16, tag="enc", name=f"enc_a{k}")
                nc.scalar.dma_start(ek[:], encT.ap()[k * 128:(k + 1) * 128, :])
                for q in range(4):
                    nc.tensor.matmul(
                        sc_ps[q][:1, :512],
                        lhsT=w2_sb[:, k:k + 1],
                        rhs=ek[:, q * 512:(q + 1) * 512],
                        start=(k == 0), stop=(k == 7))
            att = sp.tile([1, 2048], f32)
            for q in range(4):
                nc.vector.tensor_copy(att[:, q * 512:(q + 1) * 512],
                                      sc_ps[q][:1, :512])
            # softmax over t per b (free layout b-major: f = b*256+t)
            m8 = sp.tile([1, 8], f32)
            nc.vector.tensor_reduce(
                m8[:], att[:1, :].rearrange("p (b t) -> p b t", t=T),
                axis=X, op=ALU.max)
            for b in range(BS):
                nc.vector.tensor_scalar(
                    att[:1, b * T:(b + 1) * T], att[:1, b * T:(b + 1) * T],
                    m8[:1, b:b + 1], None, op0=ALU.subtract)
            nc.scalar.activation(att[:1, :], att[:1, :], AF.Exp)
            s8 = sp.tile([1, 8], f32)
            nc.vector.tensor_reduce(
                s8[:], att[:1, :].rearrange("p (b t) -> p b t", t=T),
                axis=X, op=ALU.add)
            # e = exp(s-m) transposed to [t, b] chunks (bf16) for PE context
            attn_T0 = sp.tile([128, 8], bf16)
            attn_T1 = sp.tile([128, 8], bf16)
            for b in range(BS):
                for th, atile in ((0, attn_T0), (1, attn_T1)):
                    trp = psA.tile([128, 512], f32, space="PSUM", tag="ps",
                                   name=f"tra_{b}_{th}")
                    nc.tensor.transpose(
                        trp[:, :1], att[:1, b * T + th * 128:
                                        b * T + (th + 1) * 128],
                        ident[:1, :1])
                    nc.vector.tensor_copy(atile[:, b:b + 1], trp[:, :1])
            r8 = sp.tile([1, 8], f32)
            nc.vector.reciprocal(r8[:], s8[:])
            # context via per-b matmuls over t, kept flat on one partition
            ctxf = sp.tile([1, BS * H], f32)
            for b in range(BS):
                e2a = enc2p.tile([128, H], bf16, tag="e2", name=f"e2a_{b}")
                nc.scalar.dma_start(e2a[:], enc2.ap()[0:128, b * H:(b + 1) * H])
                e2b = enc2p.tile([128, H], bf16, tag="e2", name=f"e2b_{b}")
                nc.scalar.dma_start(e2b[:], enc2.ap()[128:256, b * H:(b + 1) * H])
                for nh in range(2):
                    cps = psA.tile([128, 512], f32, space="PSUM", tag="ps",
                                   name=f"cps_{b}_{nh}")
                    for tch, (e2c, atile) in enumerate(
                            ((e2a, attn_T0), (e2b, attn_T1))):
                        nc.tensor.matmul(
                            cps[:1, :512],
                            lhsT=atile[:, b:b + 1],
                            rhs=e2c[:, nh * 512:(nh + 1) * 512],
                            start=(tch == 0), stop=(tch == 1))
                    nc.vector.tensor_copy(
                        ctxf[:1, b * H + nh * 512: b * H + (nh + 1) * 512],
                        cps[:1, :512])
                # normalize this b's row by 1/sum
                nc.vector.tensor_scalar(
                    ctxf[:1, b * H:(b + 1) * H], ctxf[:1, b * H:(b + 1) * H],
                    r8[:1, b:b + 1], None, op0=ALU.mult)

            # ============ AllGather context rows ==========================
            ag2_in = dr.tile([BS, H], f32)
            nc.sync.dma_start(ag2_in[:], ctxf[:1, :])
            ag2_out = dr.tile([B, H], f32, addr_space="Shared")
            nc.gpsimd.collective_compute(
                "AllGather", ALU.bypass, replica_groups=rg,
                ins=[ag2_in.opt()], outs=[ag2_out.opt()])
            cf_sb = sp.tile([B, H], f32)
            nc.sync.dma_start(cf_sb[:], ag2_out[:])
            # ctx^T chunks: xt slots 4-11 (f32r) + bf16 copies for vocab mm
            xtb = sp.tile([128, 8 * B], bf16)
            for j in range(8):
                trp = psA.tile([128, 512], f32, space="PSUM", tag="ps",
                               name=f"trx_{j}")
                nc.tensor.transpose(trp[:, :B], cf_sb[:, j * 128:(j + 1) * 128],
                                    ident[:B, :B])
                nc.vector.tensor_copy(xt_sb[:, (4 + j) * B:(5 + j) * B],
                                      trp[:, :B])
                nc.vector.tensor_copy(xtb[:, j * B:(j + 1) * B], trp[:, :B])

            # emb -> x^T chunks 0-3 (after attention work in queue order)
            emb_sb = sp.tile([B, IN], f32)
            nc.sync.dma_start(emb_sb[:], ar_out[:])
            for j in range(4):
                trp = psA.tile([128, 512], f32, space="PSUM", tag="ps",
                               name=f"tre_{j}")
                nc.tensor.transpose(trp[:, :B], emb_sb[:, j * 128:(j + 1) * 128],
                                    ident[:B, :B])
                nc.vector.tensor_copy(xt_sb[:, j * B:(j + 1) * B], trp[:, :B])

            # ================= GRU (hidden-sharded) ======================
            g_r = psA.tile([128, 512], f32, space="PSUM", tag="ps")
            g_z = psA.tile([128, 512], f32, space="PSUM", tag="ps")
            g_in = psA.tile([128, 512], f32, space="PSUM", tag="ps")
            g_hn = psA.tile([128, 512], f32, space="PSUM", tag="ps")
            for k in range(12):
                xk = xt_sb[:, k * B:(k + 1) * B]
                for gi, ps in ((0, g_r), (1, g_z), (2, g_in)):
                    nc.tensor.matmul(
                        ps[:HS, :B],
                        lhsT=wih_sb[:, k * 3 * HS + gi * HS: k * 3 * HS + (gi + 1) * HS],
                        rhs=xk, start=(k == 0),
                        stop=(gi == 2 and k == 11))
            for k in range(8):
                hk = lhT_sb[:, k * B:(k + 1) * B]
                for gi, ps, st in ((0, g_r, False), (1, g_z, False), (2, g_hn, k == 0)):
                    nc.tensor.matmul(
                        ps[:HS, :B],
                        lhsT=whh_sb[:, k * 3 * HS + gi * HS:
                                    k * 3 * HS + (gi + 1) * HS],
                        rhs=hk, start=st, stop=(k == 7))
            # r = sigmoid(g_r + b_r) = 1/(1+exp(-g_r - b_r))
            r_sb = sp.tile([HS, B], f32)
            nc.scalar.activation(r_sb[:], g_r[:HS, :B], AF.Exp,
                                 bias=gb_sb[:, 0:1], scale=-1.0)
            nc.vector.tensor_scalar_add(r_sb[:], r_sb[:], 1.0)
            nc.vector.reciprocal(r_sb[:], r_sb[:])
            z_sb = sp.tile([HS, B], f32)
            nc.scalar.activation(z_sb[:], g_z[:HS, :B], AF.Exp,
                                 bias=gb_sb[:, 1:2], scale=-1.0)
            nc.vector.tensor_scalar_add(z_sb[:], z_sb[:], 1.0)
            nc.vector.reciprocal(z_sb[:], z_sb[:])
            hn_sb = sp.tile([HS, B], f32)
            nc.vector.tensor_scalar(hn_sb[:], g_hn[:HS, :B], gb_sb[:, 3:4],
                                    None, op0=ALU.add)
            t2 = sp.tile([HS, B], f32)
            nc.vector.tensor_tensor(out=t2[:], in0=r_sb[:], in1=hn_sb[:],
                                    op=ALU.mult)
            nc.vector.tensor_tensor(out=t2[:], in0=t2[:], in1=g_in[:HS, :B],
                                    op=ALU.add)
            # n = tanh(t2 + b_in) = 2/(1+exp(-2*t2 - 2*b_in)) - 1
            n_sb = sp.tile([HS, B], f32)
            nc.scalar.activation(n_sb[:], t2[:], AF.Exp,
                                 bias=gb_sb[:, 2:3], scale=-2.0)
            nc.vector.tensor_scalar_add(n_sb[:], n_sb[:], 1.0)
            nc.vector.reciprocal(n_sb[:], n_sb[:])
            nc.vector.tensor_scalar(n_sb[:], n_sb[:], 2.0, -1.0,
                                    op0=ALU.mult, op1=ALU.add)
            # h' = n + z*(h - n)
            hT = sp.tile([HS, B], f32)
            nc.vector.tensor_tensor(out=hT[:], in0=lhsl_sb[:], in1=n_sb[:],
                                    op=ALU.subtract)
            nc.vector.tensor_tensor(out=hT[:], in0=z_sb[:], in1=hT[:],
                                    op=ALU.mult)
            nc.vector.tensor_tensor(out=hT[:], in0=n_sb[:], in1=hT[:],
                                    op=ALU.add)
            ag3_in = dr.tile([HS, B], f32)
            nc.sync.dma_start(ag3_in[:], hT[:])
            ag3_out = dr.tile([H, B], f32, addr_space="Shared")
            nc.gpsimd.collective_compute(
                "AllGather", ALU.bypass, replica_groups=rg,
                ins=[ag3_in.opt()], outs=[ag3_out.opt()])
            nc.gpsimd.dma_start(hnT_o[:], ag3_out[:])
            # h_new^T chunks for the vocab matmul (f32 staging + bf16 cast)
            ynf = sp.tile([128, 8 * B], f32)
            nc.sync.dma_start(
                ynf[:], ag3_out[:].rearrange("(k p) b -> p k b", p=128))
            ynT = sp.tile([128, 8 * B], bf16)
            nc.vector.tensor_copy(ynT[:], ynf[:])

            # ================= vocab projection ==========================
            def lhsT_for(k):
                if k < 8:
                    return ynT[:, k * B:(k + 1) * B]          # h_new rows
                return xtb[:, (k - 8) * B:(k - 7) * B]        # ctx rows

            logits = lp.tile([B, VS], f32)
            mx = sp.tile([B, 16], f32)
            msec = sp.tile([B, 4], f32)
            ssec = sp.tile([B, 4], f32)
            nvt_total = 0
            for s, (soff, swid) in enumerate(zip(SEC_OFF, SECS)):
                nvt = (swid + 511) // 512
                ps_v = []
                for vv in range(nvt):
                    ps_v.append(psB.tile([B, 512], f32, space="PSUM",
                                         tag="mm", name=f"mmps_{s}_{vv}"))
                brow = bp.tile([1, 2048], bf16, tag="brow", name=f"brow_{s}")
                nc.sync.dma_start(brow[:1, :swid],
                                  owT.ap()[K2H:K2H + 1, soff:soff + swid])
                # context k-chunks first (ready earlier), then h_new, bias
                korder = list(range(8, 16)) + list(range(8))
                for ki, k in enumerate(korder):
                    wt = wp.tile([128, 2048], bf16, tag="w",
                                 name=f"wt_{s}_{k}")
                    nc.sync.dma_start(wt[:, :swid],
                                      owT.ap()[k * 128:(k + 1) * 128,
                                               soff:soff + swid])
                    for vv in range(nvt):
                        w0 = vv * 512
                        w1 = min(swid, w0 + 512)
                        nc.tensor.matmul(ps_v[vv][:, :w1 - w0],
                                         lhsT=lhsT_for(k),
                                         rhs=wt[:, w0:w1],
                                         start=(ki == 0), stop=False)
                for vv in range(nvt):
                    w0 = vv * 512
                    w1 = min(swid, w0 + 512)
                    nc.tensor.matmul(ps_v[vv][:, :w1 - w0],
                                     lhsT=ones_b[:1, :B],
                                     rhs=brow[:1, w0:w1],
                                     start=False, stop=True)
                sv0 = nvt_total
                for vv in range(nvt):
                    w0 = vv * 512
                    w1 = min(swid, w0 + 512)
                    nc.vector.tensor_copy(logits[:, soff + w0:soff + w1],
                                          ps_v[vv][:, :w1 - w0])
                    nc.vector.tensor_reduce(mx[:, nvt_total:nvt_total + 1],
                                            ps_v[vv][:, :w1 - w0],
                                            axis=X, op=ALU.max)
                    nvt_total += 1
                # per-section partial logsumexp, overlapped with next section
                nc.vector.tensor_reduce(msec[:, s:s + 1], mx[:, sv0:nvt_total],
                                        axis=X, op=ALU.max)
                negms = sp.tile([B, 1], f32, tag="negms", name=f"negms_{s}")
                nc.vector.tensor_scalar(negms[:], msec[:, s:s + 1], -1.0,
                                        None, op0=ALU.mult)
                esc = scr.tile([B, 2048], f32, tag="esc", name=f"esc_{s}")
                nc.scalar.activation(esc[:, :swid], logits[:, soff:soff + swid],
                                     AF.Exp, bias=negms[:, :1], scale=1.0,
                                     accum_out=ssec[:, s:s + 1])

            # combine sections: m_c = max_s msec, s_c = sum ssec*exp(msec-m_c)
            m_c = sp.tile([B, 1], f32)
            nc.vector.tensor_reduce(m_c[:], msec[:, :4], axis=X, op=ALU.max)
            d4 = sp.tile([B, 4], f32)
            nc.vector.tensor_scalar(d4[:], msec[:, :4], m_c[:, :1], None,
                                    op0=ALU.subtract)
            nc.scalar.activation(d4[:], d4[:], AF.Exp)
            nc.vector.tensor_tensor(out=d4[:], in0=d4[:], in1=ssec[:, :4],
                                    op=ALU.mult)
            s_c = sp.tile([B, 1], f32)
            nc.vector.tensor_reduce(s_c[:], d4[:, :4], axis=X, op=ALU.add)
            ms = sp.tile([B, 2], f32)
            nc.vector.tensor_copy(ms[:, 0:1], m_c[:])
            nc.vector.tensor_copy(ms[:, 1:2], s_c[:])
            ag4_in = dr.tile([B, 2], f32)
            nc.sync.dma_start(ag4_in[:], ms[:])
            ag4_out = dr.tile([NCORES * B, 2], f32, addr_space="Shared")
            nc.gpsimd.collective_compute(
                "AllGather", ALU.bypass, replica_groups=rg,
                ins=[ag4_in.opt()], outs=[ag4_out.opt()])
            # land j-major: msall[:, 0:8] = per-rank maxes, [:, 8:16] = sums
            msall = sp.tile([B, 16], f32)
            nc.sync.dma_start(
                msall[:, :].rearrange("p (j r) -> p j r", r=NCORES),
                ag4_out[:].rearrange("(r p) j -> p j r", p=B))
            mg = sp.tile([B, 1], f32)
            nc.vector.tensor_reduce(mg[:], msall[:, 0:8], axis=X, op=ALU.max)
            d8 = sp.tile([B, 8], f32)
            nc.vector.tensor_scalar(
                d8[:], msall[:, 0:8], mg[:, :1], None, op0=ALU.subtract)
            nc.scalar.activation(d8[:], d8[:], AF.Exp)
            nc.vector.tensor_tensor(
                out=d8[:], in0=d8[:], in1=msall[:, 8:16], op=ALU.mult)
            sg = sp.tile([B, 1], f32)
            nc.vector.tensor_reduce(sg[:], d8[:], axis=X, op=ALU.add)
            nc.scalar.activation(sg[:], sg[:], AF.Ln)
            g_sb = sp.tile([B, 1], f32)
            nc.vector.tensor_tensor(out=g_sb[:], in0=mg[:], in1=sg[:],
                                    op=ALU.add)
            # logp = logits - g, stream out in halves
            for hhalf in range(2):
                c0 = hhalf * (VS // 2)
                c1 = VS if hhalf else VS // 2
                nc.vector.tensor_scalar(logits[:, c0:c1], logits[:, c0:c1],
                                        g_sb[:, :1], None, op0=ALU.subtract)
                nc.sync.dma_start(logp_o.ap()[:, c0:c1], logits[:, c0:c1])

    nc.compile()
    return nc


def _prep_inputs(inputs):
    last_output = np.asarray(inputs["last_output"]).reshape(B)
    last_hidden = np.asarray(inputs["last_hidden"], dtype=np.float32)
    enc = np.asarray(inputs["encoder_outputs"], dtype=np.float32)
    emb = np.asarray(inputs["embedding"], dtype=np.float32)
    attn_W = np.asarray(inputs["attn_W"], dtype=np.float32)
    v = np.asarray(inputs["v"], dtype=np.float32)
    W_ih = np.asarray(inputs["W_ih"], dtype=np.float32)
    W_hh = np.asarray(inputs["W_hh"], dtype=np.float32)
    b_ih = np.asarray(inputs["b_ih"], dtype=np.float32)
    b_hh = np.asarray(inputs["b_hh"], dtype=np.float32)
    out_W = np.asarray(inputs["out_W"], dtype=np.float32)
    out_b = np.asarray(inputs["out_b"], dtype=np.float32)

    w2 = (attn_W[:, H:].astype(np.float64).T @ v.astype(np.float64))
    w2 = np.ascontiguousarray(
        w2.astype(ml_dtypes.bfloat16).reshape(8, 128).T)   # [128, 8]
    lhT = np.ascontiguousarray(last_hidden.T)              # [1024, 64]
    lhTs = np.ascontiguousarray(
        lhT.reshape(8, 128, B).transpose(1, 0, 2).reshape(128, 8 * B))

    in_maps = []
    for c in range(NCORES):
        bc = slice(BS * c, BS * (c + 1))
        encT_c = np.ascontiguousarray(
            enc[:, bc, :].transpose(2, 1, 0)).reshape(H, BS * T).astype(
                ml_dtypes.bfloat16)
        enc2_c = np.ascontiguousarray(enc[:, bc, :]).reshape(
            T, BS * H).astype(ml_dtypes.bfloat16)
        rows = np.r_[HS * c:HS * (c + 1),
                     H + HS * c:H + HS * (c + 1),
                     2 * H + HS * c:2 * H + HS * (c + 1)]
        wihT_c = W_ih[rows, :].T                           # [1536, 384]
        wihT_c = np.ascontiguousarray(
            wihT_c.reshape(12, 128, 3 * HS).transpose(1, 0, 2).reshape(
                128, 12 * 3 * HS))
        whhT_c = W_hh[rows, :].T                           # [1024, 384]
        whhT_c = np.ascontiguousarray(
            whhT_c.reshape(8, 128, 3 * HS).transpose(1, 0, 2).reshape(
                128, 8 * 3 * HS))
        br = (b_ih + b_hh)[rows]
        gbias_c = np.stack([-br[:HS], -br[HS:2 * HS],
                            -2.0 * b_ih[rows][2 * HS:],
                            b_hh[rows][2 * HS:]], axis=1)
        gbias_c = np.ascontiguousarray(gbias_c.astype(np.float32))
        loc = last_output.astype(np.int64) - VS * c
        loc = np.where((loc >= 0) & (loc < VS), loc, VS).astype(np.int32)
        vs = slice(VS * c, VS * (c + 1))
        owT_c = np.ascontiguousarray(
            np.vstack([out_W[vs].T, out_b[vs][None, :]]).astype(
                ml_dtypes.bfloat16))
        in_maps.append({
            "encT": encT_c,
            "enc2": enc2_c,
            "w2": w2,
            "emb_tab": np.ascontiguousarray(emb[vs]),
            "idx": loc[:, None],
            "lhT": lhTs,
            "lh_sl": np.ascontiguousarray(lhT[HS * c:HS * (c + 1)]),
            "wihT": wihT_c,
            "whhT": whhT_c,
            "gbias": gbias_c,
            "owT": owT_c,
        })
    return in_maps


LAST_EXEC_TIME_NS = None
LAST_PROFILE = None


def kernel(**inputs):
    global LAST_EXEC_TIME_NS, LAST_PROFILE
    if "nc" not in _CACHED:
        _CACHED["nc"] = _build()
    nc = _CACHED["nc"]
    in_maps = _prep_inputs(inputs)
    trace = bool(int(os.environ.get("BASS_KERNEL_TRACE", "0")))
    res = run_bass_kernel_spmd(nc, in_maps, list(range(NCORES)), trace=trace)
    LAST_EXEC_TIME_NS = res.exec_time_ns
    LAST_PROFILE = res.profile_json
    r = res.results
    logp = np.concatenate([r[c]["logp"] for c in range(NCORES)], axis=1)
    h_new = np.ascontiguousarray(r[0]["hnT"].T)
    return logp.astype(np.float32), h_new.astype(np.float32)


# revision 18
# speedup vs baseline: 1.1418x; 1.1418x over previous
"""Trainium2 Bass kernel for a GRU attention-decoder step, 8-way sharded.

Sharding:
  - Attention + context: data-parallel over batch (8 rows/core), with the
    algebraic collapse scores = enc . (attn_W[:,H:]^T v)  (the last_hidden
    part of the Bahdanau energy is constant over t and cancels in softmax).
  - Embedding: vocab-sharded table, masked indirect-DMA gather + AllReduce.
  - GRU: hidden-sharded (128 dims/core), AllGather of context rows and
    h_new^T.
  - Vocab projection: out_W row-sharded (6250 rows/core), bf16 weight
    stream + f32 psum accumulate, log-softmax via AllGather of per-core
    (max, sumexp) with per-section partial sums overlapped with the stream.

Self-contained: hardcodes all shapes; builds+compiles the Bass program once
per process and runs it on cores 0-7 via run_bass_kernel_spmd.
"""
import os
import numpy as np
import ml_dtypes

import concourse.bass as bass
import concourse.tile as tile
from concourse import bacc, mybir
from concourse.bass_utils import run_bass_kernel_spmd
from concourse.masks import make_identity

V, IN, H, B, T = 50000, 512, 1024, 64, 256
NCORES = 8
VS = V // NCORES          # 6250 vocab rows per core
BS = B // NCORES          # 8 batch rows per core
HS = H // NCORES          # 128 hidden dims per core
K2H = 2 * H               # 2048 contraction dim of vocab matmul
KX = IN + H               # 1536 GRU input width

f32 = mybir.dt.float32
f32r = mybir.dt.float32r
bf16 = mybir.dt.bfloat16
i32 = mybir.dt.int32
AF = mybir.ActivationFunctionType
ALU = mybir.AluOpType
X = mybir.AxisListType.X

SECS = [2048, 2048, 2048, 106]
SEC_OFF = [0, 2048, 4096, 6144]

_CACHED = {}


def _build():
    nc = bacc.Bacc("TRN2", target_bir_lowering=False, debug=False,
                   num_devices=NCORES)
    rg = [list(range(NCORES))]

    # ---- per-core external inputs ----
    encT = nc.dram_tensor("encT", [H, BS * T], bf16, kind="ExternalInput")
    enc2 = nc.dram_tensor("enc2", [T, BS * H], bf16, kind="ExternalInput")
    w2 = nc.dram_tensor("w2", [128, 8], bf16, kind="ExternalInput")
    emb_tab = nc.dram_tensor("emb_tab", [VS, IN], f32, kind="ExternalInput")
    idx = nc.dram_tensor("idx", [B, 1], i32, kind="ExternalInput")
    lhT = nc.dram_tensor("lhT", [128, 8 * B], bf16, kind="ExternalInput")
    lh_sl = nc.dram_tensor("lh_sl", [HS, B], f32, kind="ExternalInput")
    wihT = nc.dram_tensor("wihT", [128, 12 * 3 * HS], bf16, kind="ExternalInput")
    whhT = nc.dram_tensor("whhT", [128, 8 * 3 * HS], bf16, kind="ExternalInput")
    gbias = nc.dram_tensor("gbias", [HS, 4], f32, kind="ExternalInput")
    owT = nc.dram_tensor("owT", [K2H + 1, VS], bf16, kind="ExternalInput")

    logp_o = nc.dram_tensor("logp", [B, VS], f32, kind="ExternalOutput")
    hnT_o = nc.dram_tensor("hnT", [H, B], f32, kind="ExternalOutput")

    with tile.TileContext(nc) as tc:
        with (
            tc.tile_pool(name="const", bufs=1) as cpool,
            tc.tile_pool(name="enc", bufs=3) as encp,
            tc.tile_pool(name="enc2", bufs=4) as enc2p,
            tc.tile_pool(name="small", bufs=1) as sp,
            tc.tile_pool(name="wstream", bufs=12) as wp,
            tc.tile_pool(name="brow", bufs=1) as bp,
            tc.tile_pool(name="scratch", bufs=1) as scr,
            tc.tile_pool(name="logits", bufs=1) as lp,
            tc.tile_pool(name="psA", bufs=4, space="PSUM") as psA,
            tc.tile_pool(name="psB", bufs=4, space="PSUM") as psB,
            tc.tile_pool(name="dram", bufs=1, space="DRAM") as dr,
        ):
            # ================= setup =================
            ident = cpool.tile([128, 128], f32)
            make_identity(nc, ident[:])
            ones_f = cpool.tile([1, 128], f32)
            nc.vector.memset(ones_f[:], 1.0)
            ones = cpool.tile([1, 128], f32r)
            nc.vector.tensor_copy(ones[:], ones_f[:])
            ones_b = cpool.tile([1, B], bf16)
            nc.vector.tensor_copy(ones_b[:], ones_f[:1, :B])

            # embedding gather + AllReduce first: tiny DMAs, early doorbell
            idx_sb = cpool.tile([B, 1], i32)
            nc.gpsimd.dma_start(idx_sb[:], idx[:])
            embp = sp.tile([B, IN], f32)
            nc.vector.memset(embp[:], 0.0)
            nc.gpsimd.indirect_dma_start(
                out=embp[:], out_offset=None,
                in_=emb_tab[:],
                in_offset=bass.IndirectOffsetOnAxis(ap=idx_sb[:, :1], axis=0),
                bounds_check=VS - 1, oob_is_err=False)
            ar_in = dr.tile([B, IN], f32)
            nc.gpsimd.dma_start(ar_in[:], embp[:])
            ar_out = dr.tile([B, IN], f32, addr_space="Shared")
            nc.gpsimd.collective_compute(
                "AllReduce", ALU.add, replica_groups=rg,
                ins=[ar_in.opt()], outs=[ar_out.opt()])

            # dense setup DMAs (host pre-swizzled to [128, free] layouts)
            w2_sb = cpool.tile([128, 8], bf16)
            nc.gpsimd.dma_start(w2_sb[:], w2.ap())
            lhT_sb = cpool.tile([128, 8 * B], bf16)
            nc.gpsimd.dma_start(lhT_sb[:], lhT.ap())
            lhsl_sb = cpool.tile([HS, B], f32)
            nc.gpsimd.dma_start(lhsl_sb[:], lh_sl[:])
            wih_sb = cpool.tile([128, 12 * 3 * HS], bf16)
            nc.gpsimd.dma_start(wih_sb[:], wihT.ap())
            whh_sb = cpool.tile([128, 8 * 3 * HS], bf16)
            nc.gpsimd.dma_start(whh_sb[:], whhT.ap())
            gb_sb = cpool.tile([HS, 4], f32)
            nc.gpsimd.dma_start(gb_sb[:], gbias[:])
            xt_sb = sp.tile([128, 12 * B], bf16)

            # ================= attention =================
            # pass 1: scores[b,t] = sum_h enc[h,b,t] * w2[h]
            sc_ps = []
            for q in range(4):
                p = psA.tile([128, 512], f32, space="PSUM", tag="ps",
                             name=f"sc_{q}")
                sc_ps.append(p)
            for k in range(8):
                ek = encp.tile([128, 2048], bf16, tag="enc", name=f"enc_a{k}")
                nc.sync.dma_start(ek[:], encT.ap()[k * 128:(k + 1) * 128, :])
                for q in range(4):
                    nc.tensor.matmul(
                        sc_ps[q][:1, :512],
                        lhsT=w2_sb[:, k:k + 1],
                        rhs=ek[:, q * 512:(q + 1) * 512],
                        start=(k == 0), stop=(k == 7))
            att = sp.tile([1, 2048], f32)
            for q in range(4):
                nc.vector.tensor_copy(att[:, q * 512:(q + 1) * 512],
                                      sc_ps[q][:1, :512])
            # softmax over t per b (free layout b-major: f = b*256+t)
            m8 = sp.tile([1, 8], f32)
            nc.vector.tensor_reduce(
                m8[:], att[:1, :].rearrange("p (b t) -> p b t", t=T),
                axis=X, op=ALU.max)
            for b in range(BS):
                nc.vector.tensor_scalar(
                    att[:1, b * T:(b + 1) * T], att[:1, b * T:(b + 1) * T],
                    m8[:1, b:b + 1], None, op0=ALU.subtract)
            nc.scalar.activation(att[:1, :], att[:1, :], AF.Exp)
            s8 = sp.tile([1, 8], f32)
            nc.vector.tensor_reduce(
                s8[:], att[:1, :].rearrange("p (b t) -> p b t", t=T),
                axis=X, op=ALU.add)
            # e = exp(s-m) transposed to [t, b] chunks (bf16) for PE context
            attn_T0 = sp.tile([128, 8], bf16)
            attn_T1 = sp.tile([128, 8], bf16)
            for b in range(BS):
                for th, atile in ((0, attn_T0), (1, attn_T1)):
                    trp = psA.tile([128, 512], f32, space="PSUM", tag="ps",
                                   name=f"tra_{b}_{th}")
                    nc.tensor.transpose(
                        trp[:, :1], att[:1, b * T + th * 128:
                                        b * T + (th + 1) * 128],
                        ident[:1, :1])
                    nc.vector.tensor_copy(atile[:, b:b + 1], trp[:, :1])
            r8 = sp.tile([1, 8], f32)
            nc.vector.reciprocal(r8[:], s8[:])
            # context via per-b matmuls over t, kept flat on one partition
            ctxf = sp.tile([1, BS * H], f32)
            for b in range(BS):
                e2a = enc2p.tile([128, H], bf16, tag="e2", name=f"e2a_{b}")
                nc.sync.dma_start(e2a[:], enc2.ap()[0:128, b * H:(b + 1) * H])
                e2b = enc2p.tile([128, H], bf16, tag="e2", name=f"e2b_{b}")
                nc.sync.dma_start(e2b[:], enc2.ap()[128:256, b * H:(b + 1) * H])
                for nh in range(2):
                    cps = psA.tile([128, 512], f32, space="PSUM", tag="ps",
                                   name=f"cps_{b}_{nh}")
                    for tch, (e2c, atile) in enumerate(
                            ((e2a, attn_T0), (e2b, attn_T1))):
                        nc.tensor.matmul(
                            cps[:1, :512],
                            lhsT=atile[:, b:b + 1],
                            rhs=e2c[:, nh * 512:(nh + 1) * 512],
                            start=(tch == 0), stop=(tch == 1))
                    nc.vector.tensor_copy(
                        ctxf[:1, b * H + nh * 512: b * H + (nh + 1) * 512],
                        cps[:1, :512])
                # normalize this b's row by 1/sum
                nc.vector.tensor_scalar(
                    ctxf[:1, b * H:(b + 1) * H], ctxf[:1, b * H:(b + 1) * H],
                    r8[:1, b:b + 1], None, op0=ALU.mult)

            # ============ AllGather context rows ==========================
            ag2_in = dr.tile([BS, H], f32)
            nc.scalar.dma_start(ag2_in[:], ctxf[:1, :])
            ag2_out = dr.tile([B, H], f32, addr_space="Shared")
            nc.gpsimd.collective_compute(
                "AllGather", ALU.bypass, replica_groups=rg,
                ins=[ag2_in.opt()], outs=[ag2_out.opt()])
            cf_sb = sp.tile([B, H], f32)
            nc.scalar.dma_start(cf_sb[:], ag2_out[:])
            # ctx^T chunks: xt slots 4-11 (bf16, GRU rhs + vocab lhsT)
            for j in range(8):
                trp = psA.tile([128, 512], f32, space="PSUM", tag="ps",
                               name=f"trx_{j}")
                nc.tensor.transpose(trp[:, :B], cf_sb[:, j * 128:(j + 1) * 128],
                                    ident[:B, :B])
                nc.vector.tensor_copy(xt_sb[:, (4 + j) * B:(5 + j) * B],
                                      trp[:, :B])

            # emb -> x^T chunks 0-3 (after attention work in queue order)
            emb_sb = sp.tile([B, IN], f32)
            nc.scalar.dma_start(emb_sb[:], ar_out[:])
            for j in range(4):
                trp = psA.tile([128, 512], f32, space="PSUM", tag="ps",
                               name=f"tre_{j}")
                nc.tensor.transpose(trp[:, :B], emb_sb[:, j * 128:(j + 1) * 128],
                                    ident[:B, :B])
                nc.vector.tensor_copy(xt_sb[:, j * B:(j + 1) * B], trp[:, :B])

            # ================= GRU (hidden-sharded) ======================
            g_r = psA.tile([128, 512], f32, space="PSUM", tag="ps")
            g_z = psA.tile([128, 512], f32, space="PSUM", tag="ps")
            g_in = psA.tile([128, 512], f32, space="PSUM", tag="ps")
            g_hn = psA.tile([128, 512], f32, space="PSUM", tag="ps")
            for k in range(12):
                xk = xt_sb[:, k * B:(k + 1) * B]
                for gi, ps in ((0, g_r), (1, g_z), (2, g_in)):
                    nc.tensor.matmul(
                        ps[:HS, :B],
                        lhsT=wih_sb[:, k * 3 * HS + gi * HS: k * 3 * HS + (gi + 1) * HS],
                        rhs=xk, start=(k == 0),
                        stop=(gi == 2 and k == 11))
            for k in range(8):
                hk = lhT_sb[:, k * B:(k + 1) * B]
                for gi, ps, st in ((0, g_r, False), (1, g_z, False), (2, g_hn, k == 0)):
                    nc.tensor.matmul(
                        ps[:HS, :B],
                        lhsT=whh_sb[:, k * 3 * HS + gi * HS:
                                    k * 3 * HS + (gi + 1) * HS],
                        rhs=hk, start=st, stop=(k == 7))
            # r = sigmoid(g_r + b_r) = 1/(1+exp(-g_r - b_r))
            r_sb = sp.tile([HS, B], f32)
            nc.scalar.activation(r_sb[:], g_r[:HS, :B], AF.Exp,
                                 bias=gb_sb[:, 0:1], scale=-1.0)
            nc.vector.tensor_scalar_add(r_sb[:], r_sb[:], 1.0)
            nc.vector.reciprocal(r_sb[:], r_sb[:])
            z_sb = sp.tile([HS, B], f32)
            nc.scalar.activation(z_sb[:], g_z[:HS, :B], AF.Exp,
                                 bias=gb_sb[:, 1:2], scale=-1.0)
            nc.vector.tensor_scalar_add(z_sb[:], z_sb[:], 1.0)
            nc.vector.reciprocal(z_sb[:], z_sb[:])
            hn_sb = sp.tile([HS, B], f32)
            nc.vector.tensor_scalar(hn_sb[:], g_hn[:HS, :B], gb_sb[:, 3:4],
                                    None, op0=ALU.add)
            t2 = sp.tile([HS, B], f32)
            nc.vector.tensor_tensor(out=t2[:], in0=r_sb[:], in1=hn_sb[:],
                                    op=ALU.mult)
            nc.vector.tensor_tensor(out=t2[:], in0=t2[:], in1=g_in[:HS, :B],
                                    op=ALU.add)
            # n = tanh(t2 + b_in) = 2/(1+exp(-2*t2 - 2*b_in)) - 1
            n_sb = sp.tile([HS, B], f32)
            nc.scalar.activation(n_sb[:], t2[:], AF.Exp,
                                 bias=gb_sb[:, 2:3], scale=-2.0)
            nc.vector.tensor_scalar_add(n_sb[:], n_sb[:], 1.0)
            nc.vector.reciprocal(n_sb[:], n_sb[:])
            nc.vector.tensor_scalar(n_sb[:], n_sb[:], 2.0, -1.0,
                                    op0=ALU.mult, op1=ALU.add)
            # h' = n + z*(h - n)
            hT = sp.tile([HS, B], f32)
            nc.vector.tensor_tensor(out=hT[:], in0=lhsl_sb[:], in1=n_sb[:],
                                    op=ALU.subtract)
            nc.vector.tensor_tensor(out=hT[:], in0=z_sb[:], in1=hT[:],
                                    op=ALU.mult)
            nc.vector.tensor_tensor(out=hT[:], in0=n_sb[:], in1=hT[:],
                                    op=ALU.add)
            ag3_in = dr.tile([HS, B], f32)
            nc.scalar.dma_start(ag3_in[:], hT[:])
            ag3_out = dr.tile([H, B], f32, addr_space="Shared")
            nc.gpsimd.collective_compute(
                "AllGather", ALU.bypass, replica_groups=rg,
                ins=[ag3_in.opt()], outs=[ag3_out.opt()])
            nc.gpsimd.dma_start(hnT_o[:], ag3_out[:])
            # h_new^T chunks for the vocab matmul (f32 staging + bf16 cast)
            ynf = sp.tile([128, 8 * B], f32)
            nc.scalar.dma_start(
                ynf[:], ag3_out[:].rearrange("(k p) b -> p k b", p=128))
            ynT = sp.tile([128, 8 * B], bf16)
            nc.vector.tensor_copy(ynT[:], ynf[:])

            # ================= vocab projection ==========================
            def lhsT_for(k):
                if k < 8:
                    return ynT[:, k * B:(k + 1) * B]          # h_new rows
                return xt_sb[:, (k - 4) * B:(k - 3) * B]      # ctx rows

            logits = lp.tile([B, VS], f32)
            mx = sp.tile([B, 16], f32)
            msec = sp.tile([B, 4], f32)
            ssec = sp.tile([B, 4], f32)
            nvt_total = 0
            for s, (soff, swid) in enumerate(zip(SEC_OFF, SECS)):
                nvt = (swid + 511) // 512
                ps_v = []
                for vv in range(nvt):
                    ps_v.append(psB.tile([B, 512], f32, space="PSUM",
                                         tag="mm", name=f"mmps_{s}_{vv}"))
                brow = bp.tile([1, 2048], bf16, tag="brow", name=f"brow_{s}")
                nc.sync.dma_start(brow[:1, :swid],
                                  owT.ap()[K2H:K2H + 1, soff:soff + swid])
                # context k-chunks first (ready earlier), then h_new, bias
                korder = list(range(8, 16)) + list(range(8))
                for ki, k in enumerate(korder):
                    wt = wp.tile([128, 2048], bf16, tag="w",
                                 name=f"wt_{s}_{k}")
                    nc.sync.dma_start(wt[:, :swid],
                                      owT.ap()[k * 128:(k + 1) * 128,
                                               soff:soff + swid])
                    for vv in range(nvt):
                        w0 = vv * 512
                        w1 = min(swid, w0 + 512)
                        nc.tensor.matmul(ps_v[vv][:, :w1 - w0],
                                         lhsT=lhsT_for(k),
                                         rhs=wt[:, w0:w1],
                                         start=(ki == 0), stop=False)
                for vv in range(nvt):
                    w0 = vv * 512
                    w1 = min(swid, w0 + 512)
                    nc.tensor.matmul(ps_v[vv][:, :w1 - w0],
                                     lhsT=ones_b[:1, :B],
                                     rhs=brow[:1, w0:w1],
                                     start=False, stop=True)
                sv0 = nvt_total
                for vv in range(nvt):
                    w0 = vv * 512
                    w1 = min(swid, w0 + 512)
                    nc.vector.tensor_copy(logits[:, soff + w0:soff + w1],
                                          ps_v[vv][:, :w1 - w0])
                    nc.vector.tensor_reduce(mx[:, nvt_total:nvt_total + 1],
                                            ps_v[vv][:, :w1 - w0],
                                            axis=X, op=ALU.max)
                    nvt_total += 1
                # per-section partial logsumexp, overlapped with next section
                nc.vector.tensor_reduce(msec[:, s:s + 1], mx[:, sv0:nvt_total],
                                        axis=X, op=ALU.max)
                negms = sp.tile([B, 1], f32, tag="negms", name=f"negms_{s}")
                nc.vector.tensor_scalar(negms[:], msec[:, s:s + 1], -1.0,
                                        None, op0=ALU.mult)
                esc = scr.tile([B, 2048], f32, tag="esc", name=f"esc_{s}")
                nc.scalar.activation(esc[:, :swid], logits[:, soff:soff + swid],
                                     AF.Exp, bias=negms[:, :1], scale=1.0,
                                     accum_out=ssec[:, s:s + 1])

            # combine sections: m_c = max_s msec, s_c = sum ssec*exp(msec-m_c)
            m_c = sp.tile([B, 1], f32)
            nc.vector.tensor_reduce(m_c[:], msec[:, :4], axis=X, op=ALU.max)
            d4 = sp.tile([B, 4], f32)
            nc.vector.tensor_scalar(d4[:], msec[:, :4], m_c[:, :1], None,
                                    op0=ALU.subtract)
            nc.scalar.activation(d4[:], d4[:], AF.Exp)
            nc.vector.tensor_tensor(out=d4[:], in0=d4[:], in1=ssec[:, :4],
                                    op=ALU.mult)
            s_c = sp.tile([B, 1], f32)
            nc.vector.tensor_reduce(s_c[:], d4[:, :4], axis=X, op=ALU.add)
            ms = sp.tile([B, 2], f32)
            nc.vector.tensor_copy(ms[:, 0:1], m_c[:])
            nc.vector.tensor_copy(ms[:, 1:2], s_c[:])
            ag4_in = dr.tile([B, 2], f32)
            nc.scalar.dma_start(ag4_in[:], ms[:])
            ag4_out = dr.tile([NCORES * B, 2], f32, addr_space="Shared")
            nc.gpsimd.collective_compute(
                "AllGather", ALU.bypass, replica_groups=rg,
                ins=[ag4_in.opt()], outs=[ag4_out.opt()])
            # land j-major: msall[:, 0:8] = per-rank maxes, [:, 8:16] = sums
            msall = sp.tile([B, 16], f32)
            nc.scalar.dma_start(
                msall[:, :].rearrange("p (j r) -> p j r", r=NCORES),
                ag4_out[:].rearrange("(r p) j -> p j r", p=B))
            mg = sp.tile([B, 1], f32)
            nc.vector.tensor_reduce(mg[:], msall[:, 0:8], axis=X, op=ALU.max)
            d8 = sp.tile([B, 8], f32)
            nc.vector.tensor_scalar(
                d8[:], msall[:, 0:8], mg[:, :1], None, op0=ALU.subtract)
            nc.scalar.activation(d8[:], d8[:], AF.Exp)
            nc.vector.tensor_tensor(
                out=d8[:], in0=d8[:], in1=msall[:, 8:16], op=ALU.mult)
            sg = sp.tile([B, 1], f32)
            nc.vector.tensor_reduce(sg[:], d8[:], axis=X, op=ALU.add)
            nc.scalar.activation(sg[:], sg[:], AF.Ln)
            g_sb = sp.tile([B, 1], f32)
            nc.vector.tensor_tensor(out=g_sb[:], in0=mg[:], in1=sg[:],
                                    op=ALU.add)
            # logp = logits - g, stream out in halves
            for hhalf in range(2):
                c0 = hhalf * (VS // 2)
                c1 = VS if hhalf else VS // 2
                nc.vector.tensor_scalar(logits[:, c0:c1], logits[:, c0:c1],
                                        g_sb[:, :1], None, op0=ALU.subtract)
                nc.scalar.dma_start(logp_o.ap()[:, c0:c1], logits[:, c0:c1])

    nc.compile()
    return nc


def _prep_inputs(inputs):
    last_output = np.asarray(inputs["last_output"]).reshape(B)
    last_hidden = np.asarray(inputs["last_hidden"], dtype=np.float32)
    enc = np.asarray(inputs["encoder_outputs"], dtype=np.float32)
    emb = np.asarray(inputs["embedding"], dtype=np.float32)
    attn_W = np.asarray(inputs["attn_W"], dtype=np.float32)
    v = np.asarray(inputs["v"], dtype=np.float32)
    W_ih = np.asarray(inputs["W_ih"], dtype=np.float32)
    W_hh = np.asarray(inputs["W_hh"], dtype=np.float32)
    b_ih = np.asarray(inputs["b_ih"], dtype=np.float32)
    b_hh = np.asarray(inputs["b_hh"], dtype=np.float32)
    out_W = np.asarray(inputs["out_W"], dtype=np.float32)
    out_b = np.asarray(inputs["out_b"], dtype=np.float32)

    w2 = (attn_W[:, H:].astype(np.float64).T @ v.astype(np.float64))
    w2 = np.ascontiguousarray(
        w2.astype(ml_dtypes.bfloat16).reshape(8, 128).T)   # [128, 8]
    lhT = np.ascontiguousarray(last_hidden.T)              # [1024, 64]
    lhTs = np.ascontiguousarray(
        lhT.reshape(8, 128, B).transpose(1, 0, 2).reshape(128, 8 * B).astype(
            ml_dtypes.bfloat16))

    in_maps = []
    for c in range(NCORES):
        bc = slice(BS * c, BS * (c + 1))
        encT_c = np.ascontiguousarray(
            enc[:, bc, :].transpose(2, 1, 0)).reshape(H, BS * T).astype(
                ml_dtypes.bfloat16)
        enc2_c = np.ascontiguousarray(enc[:, bc, :]).reshape(
            T, BS * H).astype(ml_dtypes.bfloat16)
        rows = np.r_[HS * c:HS * (c + 1),
                     H + HS * c:H + HS * (c + 1),
                     2 * H + HS * c:2 * H + HS * (c + 1)]
        wihT_c = W_ih[rows, :].T                           # [1536, 384]
        wihT_c = np.ascontiguousarray(
            wihT_c.reshape(12, 128, 3 * HS).transpose(1, 0, 2).reshape(
                128, 12 * 3 * HS).astype(ml_dtypes.bfloat16))
        whhT_c = W_hh[rows, :].T                           # [1024, 384]
        whhT_c = np.ascontiguousarray(
            whhT_c.reshape(8, 128, 3 * HS).transpose(1, 0, 2).reshape(
                128, 8 * 3 * HS).astype(ml_dtypes.bfloat16))
        br = (b_ih + b_hh)[rows]
        gbias_c = np.stack([-br[:HS], -br[HS:2 * HS],
                            -2.0 * b_ih[rows][2 * HS:],
                            b_hh[rows][2 * HS:]], axis=1)
        gbias_c = np.ascontiguousarray(gbias_c.astype(np.float32))
        loc = last_output.astype(np.int64) - VS * c
        loc = np.where((loc >= 0) & (loc < VS), loc, VS).astype(np.int32)
        vs = slice(VS * c, VS * (c + 1))
        owT_c = np.ascontiguousarray(
            np.vstack([out_W[vs].T, out_b[vs][None, :]]).astype(
                ml_dtypes.bfloat16))
        in_maps.append({
            "encT": encT_c,
            "enc2": enc2_c,
            "w2": w2,
            "emb_tab": np.ascontiguousarray(emb[vs]),
            "idx": loc[:, None],
            "lhT": lhTs,
            "lh_sl": np.ascontiguousarray(lhT[HS * c:HS * (c + 1)]),
            "wihT": wihT_c,
            "whhT": whhT_c,
            "gbias": gbias_c,
            "owT": owT_c,
        })
    return in_maps


LAST_EXEC_TIME_NS = None
LAST_PROFILE = None


def kernel(**inputs):
    global LAST_EXEC_TIME_NS, LAST_PROFILE
    if "nc" not in _CACHED:
        _CACHED["nc"] = _build()
    nc = _CACHED["nc"]
    in_maps = _prep_inputs(inputs)
    trace = bool(int(os.environ.get("BASS_KERNEL_TRACE", "0")))
    res = run_bass_kernel_spmd(nc, in_maps, list(range(NCORES)), trace=trace)
    LAST_EXEC_TIME_NS = res.exec_time_ns
    LAST_PROFILE = res.profile_json
    r = res.results
    logp = np.concatenate([r[c]["logp"] for c in range(NCORES)], axis=1)
    h_new = np.ascontiguousarray(r[0]["hnT"].T)
    return logp.astype(np.float32), h_new.astype(np.float32)


# revision 21
# speedup vs baseline: 1.2019x; 1.0527x over previous
"""Trainium2 Bass kernel for a GRU attention-decoder step, 8-way sharded.

Sharding:
  - Attention + context: data-parallel over batch (8 rows/core), with the
    algebraic collapse scores = enc . (attn_W[:,H:]^T v)  (the last_hidden
    part of the Bahdanau energy is constant over t and cancels in softmax).
  - Embedding: vocab-sharded table, masked indirect-DMA gather + AllReduce.
  - GRU: hidden-sharded (128 dims/core), AllGather of context rows and
    h_new^T.
  - Vocab projection: out_W row-sharded (6250 rows/core), bf16 weight
    stream + f32 psum accumulate, log-softmax via AllGather of per-core
    (max, sumexp) with per-section partial sums overlapped with the stream.

Self-contained: hardcodes all shapes; builds+compiles the Bass program once
per process and runs it on cores 0-7 via run_bass_kernel_spmd.
"""
import os
import numpy as np
import ml_dtypes

import concourse.bass as bass
import concourse.tile as tile
from concourse import bacc, mybir
from concourse.bass_utils import run_bass_kernel_spmd
from concourse.masks import make_identity

V, IN, H, B, T = 50000, 512, 1024, 64, 256
NCORES = 8
VS = V // NCORES          # 6250 vocab rows per core
BS = B // NCORES          # 8 batch rows per core
HS = H // NCORES          # 128 hidden dims per core
K2H = 2 * H               # 2048 contraction dim of vocab matmul
KX = IN + H               # 1536 GRU input width

f32 = mybir.dt.float32
f32r = mybir.dt.float32r
bf16 = mybir.dt.bfloat16
i32 = mybir.dt.int32
AF = mybir.ActivationFunctionType
ALU = mybir.AluOpType
X = mybir.AxisListType.X

SECS = [2048, 2048, 2048, 106]
SEC_OFF = [0, 2048, 4096, 6144]

_CACHED = {}


def _build():
    nc = bacc.Bacc("TRN2", target_bir_lowering=False, debug=False,
                   num_devices=NCORES)
    rg = [list(range(NCORES))]

    # ---- per-core external inputs ----
    encT = nc.dram_tensor("encT", [H, BS * T], bf16, kind="ExternalInput")
    enc2 = nc.dram_tensor("enc2", [T, BS * H], bf16, kind="ExternalInput")
    w2 = nc.dram_tensor("w2", [128, 8], bf16, kind="ExternalInput")
    emb_tab = nc.dram_tensor("emb_tab", [VS, IN], f32, kind="ExternalInput")
    idx = nc.dram_tensor("idx", [B, 1], i32, kind="ExternalInput")
    lhT = nc.dram_tensor("lhT", [128, 8 * B], bf16, kind="ExternalInput")
    lh_sl = nc.dram_tensor("lh_sl", [HS, B], f32, kind="ExternalInput")
    wihT = nc.dram_tensor("wihT", [128, 12 * 3 * HS], bf16, kind="ExternalInput")
    whhT = nc.dram_tensor("whhT", [128, 8 * 3 * HS], bf16, kind="ExternalInput")
    gbias = nc.dram_tensor("gbias", [HS, 4], f32, kind="ExternalInput")
    owT = nc.dram_tensor("owT", [K2H + 1, VS], bf16, kind="ExternalInput")

    logp_o = nc.dram_tensor("logp", [B, VS], f32, kind="ExternalOutput")
    hnT_o = nc.dram_tensor("hnT", [H, B], f32, kind="ExternalOutput")

    with tile.TileContext(nc) as tc:
        with (
            tc.tile_pool(name="const", bufs=1) as cpool,
            tc.tile_pool(name="enc", bufs=3) as encp,
            tc.tile_pool(name="enc2", bufs=16) as enc2p,
            tc.tile_pool(name="small", bufs=1) as sp,
            tc.tile_pool(name="wstream", bufs=12) as wp,
            tc.tile_pool(name="brow", bufs=1) as bp,
            tc.tile_pool(name="scratch", bufs=1) as scr,
            tc.tile_pool(name="logits", bufs=1) as lp,
            tc.tile_pool(name="psA", bufs=4, space="PSUM") as psA,
            tc.tile_pool(name="psB", bufs=4, space="PSUM") as psB,
            tc.tile_pool(name="dram", bufs=1, space="DRAM") as dr,
        ):
            # ================= setup =================
            ident = cpool.tile([128, 128], f32)
            make_identity(nc, ident[:])
            ones_f = cpool.tile([1, 128], f32)
            nc.vector.memset(ones_f[:], 1.0)
            ones = cpool.tile([1, 128], f32r)
            nc.vector.tensor_copy(ones[:], ones_f[:])
            ones_b = cpool.tile([1, B], bf16)
            nc.vector.tensor_copy(ones_b[:], ones_f[:1, :B])

            # embedding gather + AllReduce first: tiny DMAs, early doorbell
            idx_sb = cpool.tile([B, 1], i32)
            nc.gpsimd.dma_start(idx_sb[:], idx[:])
            embp = sp.tile([B, IN], f32)
            nc.vector.memset(embp[:], 0.0)
            nc.gpsimd.indirect_dma_start(
                out=embp[:], out_offset=None,
                in_=emb_tab[:],
                in_offset=bass.IndirectOffsetOnAxis(ap=idx_sb[:, :1], axis=0),
                bounds_check=VS - 1, oob_is_err=False)
            ar_in = dr.tile([B, IN], f32)
            nc.gpsimd.dma_start(ar_in[:], embp[:])
            ar_out = dr.tile([B, IN], f32, addr_space="Shared")
            nc.gpsimd.collective_compute(
                "AllReduce", ALU.add, replica_groups=rg,
                ins=[ar_in.opt()], outs=[ar_out.opt()])

            # dense setup DMAs (host pre-swizzled to [128, free] layouts)
            w2_sb = cpool.tile([128, 8], bf16)
            nc.gpsimd.dma_start(w2_sb[:], w2.ap())
            lhT_sb = cpool.tile([128, 8 * B], bf16)
            nc.gpsimd.dma_start(lhT_sb[:], lhT.ap())
            lhsl_sb = cpool.tile([HS, B], f32)
            nc.gpsimd.dma_start(lhsl_sb[:], lh_sl[:])
            wih_sb = cpool.tile([128, 12 * 3 * HS], bf16)
            nc.gpsimd.dma_start(wih_sb[:], wihT.ap())
            whh_sb = cpool.tile([128, 8 * 3 * HS], bf16)
            nc.gpsimd.dma_start(whh_sb[:], whhT.ap())
            gb_sb = cpool.tile([HS, 4], f32)
            nc.gpsimd.dma_start(gb_sb[:], gbias[:])
            xt_sb = sp.tile([128, 12 * B], bf16)

            # ================= attention =================
            # pass 1: scores[b,t] = sum_h enc[h,b,t] * w2[h]
            sc_ps = []
            for qp in range(2):
                p = psA.tile([128, 512], f32, space="PSUM", tag="ps",
                             name=f"sc_{qp}")
                sc_ps.append(p)
            for k in range(8):
                ek = encp.tile([128, 2048], bf16, tag="enc", name=f"enc_a{k}")
                nc.sync.dma_start(ek[:], encT.ap()[k * 128:(k + 1) * 128, :])
                for q in range(4):
                    qp, qs = divmod(q, 2)
                    nc.tensor.matmul(
                        sc_ps[qp][64 * qs:64 * qs + 1, :512],
                        lhsT=w2_sb[:, k:k + 1],
                        rhs=ek[:, q * 512:(q + 1) * 512],
                        start=(k == 0), stop=(k == 7),
                        tile_position=(0, 64 * qs))
            att = sp.tile([1, 2048], f32)
            for q in range(4):
                qp, qs = divmod(q, 2)
                nc.vector.tensor_copy(att[:, q * 512:(q + 1) * 512],
                                      sc_ps[qp][64 * qs:64 * qs + 1, :512])
            # softmax over t per b (free layout b-major: f = b*256+t)
            m8 = sp.tile([1, 8], f32)
            nc.vector.tensor_reduce(
                m8[:], att[:1, :].rearrange("p (b t) -> p b t", t=T),
                axis=X, op=ALU.max)
            for b in range(BS):
                nc.vector.tensor_scalar(
                    att[:1, b * T:(b + 1) * T], att[:1, b * T:(b + 1) * T],
                    m8[:1, b:b + 1], None, op0=ALU.subtract)
            nc.scalar.activation(att[:1, :], att[:1, :], AF.Exp)
            s8 = sp.tile([1, 8], f32)
            nc.vector.tensor_reduce(
                s8[:], att[:1, :].rearrange("p (b t) -> p b t", t=T),
                axis=X, op=ALU.add)
            # e = exp(s-m) transposed to [t, b] chunks (bf16) for PE context
            attn_T0 = sp.tile([128, 8], bf16)
            attn_T1 = sp.tile([128, 8], bf16)
            for b in range(BS):
                for th, atile in ((0, attn_T0), (1, attn_T1)):
                    trp = psA.tile([128, 512], f32, space="PSUM", tag="ps",
                                   name=f"tra_{b}_{th}")
                    nc.tensor.transpose(
                        trp[:, :1], att[:1, b * T + th * 128:
                                        b * T + (th + 1) * 128],
                        ident[:1, :1])
                    nc.vector.tensor_copy(atile[:, b:b + 1], trp[:, :1])
            r8f = sp.tile([1, 8], f32)
            nc.vector.reciprocal(r8f[:], s8[:])
            r8 = sp.tile([1, 8], f32r)
            nc.vector.tensor_copy(r8[:], r8f[:])
            # normalize attn columns: r8 broadcast to 128 partitions via PE
            r8bc = psA.tile([128, 512], f32, space="PSUM", tag="ps",
                            name="r8bc")
            nc.tensor.matmul(r8bc[:, :8], lhsT=ones[:1, :], rhs=r8[:1, :8],
                             start=True, stop=True)
            nc.vector.tensor_tensor(out=attn_T0[:], in0=attn_T0[:],
                                    in1=r8bc[:, :8], op=ALU.mult)
            nc.vector.tensor_tensor(out=attn_T1[:], in0=attn_T1[:],
                                    in1=r8bc[:, :8], op=ALU.mult)
            # context via per-b matmuls over t (2-way col packed)
            ctxf = sp.tile([1, BS * H], f32)
            e2as, e2bs = [], []
            for b in range(BS):
                e2a = enc2p.tile([128, H], bf16, tag="e2", name=f"e2a_{b}")
                nc.sync.dma_start(e2a[:], enc2.ap()[0:128, b * H:(b + 1) * H])
                e2b = enc2p.tile([128, H], bf16, tag="e2", name=f"e2b_{b}")
                nc.sync.dma_start(e2b[:], enc2.ap()[128:256, b * H:(b + 1) * H])
                e2as.append(e2a); e2bs.append(e2b)
            for bpair in range(4):
                for nh in range(2):
                    cps = psA.tile([128, 512], f32, space="PSUM", tag="ps",
                                   name=f"cps_{bpair}_{nh}")
                    for bs_ in range(2):
                        b = 2 * bpair + bs_
                        for tch, atile in ((0, attn_T0), (1, attn_T1)):
                            e2c = (e2as if tch == 0 else e2bs)[b]
                            nc.tensor.matmul(
                                cps[64 * bs_:64 * bs_ + 1, :512],
                                lhsT=atile[:, b:b + 1],
                                rhs=e2c[:, nh * 512:(nh + 1) * 512],
                                start=(tch == 0), stop=(tch == 1),
                                tile_position=(0, 64 * bs_))
                    for bs_ in range(2):
                        b = 2 * bpair + bs_
                        nc.vector.tensor_copy(
                            ctxf[:1, b * H + nh * 512: b * H + (nh + 1) * 512],
                            cps[64 * bs_:64 * bs_ + 1, :512])

            # ============ AllGather context rows ==========================
            ag2_in = dr.tile([BS, H], f32)
            nc.scalar.dma_start(ag2_in[:], ctxf[:1, :])
            ag2_out = dr.tile([B, H], f32, addr_space="Shared")
            nc.gpsimd.collective_compute(
                "AllGather", ALU.bypass, replica_groups=rg,
                ins=[ag2_in.opt()], outs=[ag2_out.opt()])
            cf_sb = sp.tile([B, H], f32)
            nc.scalar.dma_start(cf_sb[:], ag2_out[:])
            # ctx^T chunks: xt slots 4-11 (bf16, GRU rhs + vocab lhsT)
            for j in range(8):
                trp = psA.tile([128, 512], f32, space="PSUM", tag="ps",
                               name=f"trx_{j}")
                nc.tensor.transpose(trp[:, :B], cf_sb[:, j * 128:(j + 1) * 128],
                                    ident[:B, :B])
                nc.vector.tensor_copy(xt_sb[:, (4 + j) * B:(5 + j) * B],
                                      trp[:, :B])

            # emb -> x^T chunks 0-3 (after attention work in queue order)
            emb_sb = sp.tile([B, IN], f32)
            nc.scalar.dma_start(emb_sb[:], ar_out[:])
            for j in range(4):
                trp = psA.tile([128, 512], f32, space="PSUM", tag="ps",
                               name=f"tre_{j}")
                nc.tensor.transpose(trp[:, :B], emb_sb[:, j * 128:(j + 1) * 128],
                                    ident[:B, :B])
                nc.vector.tensor_copy(xt_sb[:, j * B:(j + 1) * B], trp[:, :B])

            # ================= GRU (hidden-sharded) ======================
            g_r = psA.tile([128, 512], f32, space="PSUM", tag="ps")
            g_z = psA.tile([128, 512], f32, space="PSUM", tag="ps")
            g_in = psA.tile([128, 512], f32, space="PSUM", tag="ps")
            g_hn = psA.tile([128, 512], f32, space="PSUM", tag="ps")
            for k in range(12):
                xk = xt_sb[:, k * B:(k + 1) * B]
                for gi, ps in ((0, g_r), (1, g_z), (2, g_in)):
                    nc.tensor.matmul(
                        ps[:HS, :B],
                        lhsT=wih_sb[:, k * 3 * HS + gi * HS: k * 3 * HS + (gi + 1) * HS],
                        rhs=xk, start=(k == 0),
                        stop=(gi == 2 and k == 11))
            for k in range(8):
                hk = lhT_sb[:, k * B:(k + 1) * B]
                for gi, ps, st in ((0, g_r, False), (1, g_z, False), (2, g_hn, k == 0)):
                    nc.tensor.matmul(
                        ps[:HS, :B],
                        lhsT=whh_sb[:, k * 3 * HS + gi * HS:
                                    k * 3 * HS + (gi + 1) * HS],
                        rhs=hk, start=st, stop=(k == 7))
            # r = sigmoid(g_r + b_r) = 1/(1+exp(-g_r - b_r))
            r_sb = sp.tile([HS, B], f32)
            nc.scalar.activation(r_sb[:], g_r[:HS, :B], AF.Exp,
                                 bias=gb_sb[:, 0:1], scale=-1.0)
            nc.vector.tensor_scalar_add(r_sb[:], r_sb[:], 1.0)
            nc.vector.reciprocal(r_sb[:], r_sb[:])
            z_sb = sp.tile([HS, B], f32)
            nc.scalar.activation(z_sb[:], g_z[:HS, :B], AF.Exp,
                                 bias=gb_sb[:, 1:2], scale=-1.0)
            nc.vector.tensor_scalar_add(z_sb[:], z_sb[:], 1.0)
            nc.vector.reciprocal(z_sb[:], z_sb[:])
            hn_sb = sp.tile([HS, B], f32)
            nc.vector.tensor_scalar(hn_sb[:], g_hn[:HS, :B], gb_sb[:, 3:4],
                                    None, op0=ALU.add)
            t2 = sp.tile([HS, B], f32)
            nc.vector.tensor_tensor(out=t2[:], in0=r_sb[:], in1=hn_sb[:],
                                    op=ALU.mult)
            nc.vector.tensor_tensor(out=t2[:], in0=t2[:], in1=g_in[:HS, :B],
                                    op=ALU.add)
            # n = tanh(t2 + b_in) = 2/(1+exp(-2*t2 - 2*b_in)) - 1
            n_sb = sp.tile([HS, B], f32)
            nc.scalar.activation(n_sb[:], t2[:], AF.Exp,
                                 bias=gb_sb[:, 2:3], scale=-2.0)
            nc.vector.tensor_scalar_add(n_sb[:], n_sb[:], 1.0)
            nc.vector.reciprocal(n_sb[:], n_sb[:])
            nc.vector.tensor_scalar(n_sb[:], n_sb[:], 2.0, -1.0,
                                    op0=ALU.mult, op1=ALU.add)
            # h' = n + z*(h - n)
            hT = sp.tile([HS, B], f32)
            nc.vector.tensor_tensor(out=hT[:], in0=lhsl_sb[:], in1=n_sb[:],
                                    op=ALU.subtract)
            nc.vector.tensor_tensor(out=hT[:], in0=z_sb[:], in1=hT[:],
                                    op=ALU.mult)
            nc.vector.tensor_tensor(out=hT[:], in0=n_sb[:], in1=hT[:],
                                    op=ALU.add)
            ag3_in = dr.tile([HS, B], f32)
            nc.scalar.dma_start(ag3_in[:], hT[:])
            ag3_out = dr.tile([H, B], f32, addr_space="Shared")
            nc.gpsimd.collective_compute(
                "AllGather", ALU.bypass, replica_groups=rg,
                ins=[ag3_in.opt()], outs=[ag3_out.opt()])
            nc.gpsimd.dma_start(hnT_o[:], ag3_out[:])
            # h_new^T chunks for the vocab matmul (f32 staging + bf16 cast)
            ynf = sp.tile([128, 8 * B], f32)
            nc.scalar.dma_start(
                ynf[:], ag3_out[:].rearrange("(k p) b -> p k b", p=128))
            ynT = sp.tile([128, 8 * B], bf16)
            nc.vector.tensor_copy(ynT[:], ynf[:])

            # ================= vocab projection ==========================
            def lhsT_for(k):
                if k < 8:
                    return ynT[:, k * B:(k + 1) * B]          # h_new rows
                return xt_sb[:, (k - 4) * B:(k - 3) * B]      # ctx rows

            logits = lp.tile([B, VS], f32)
            mx = sp.tile([B, 16], f32)
            msec = sp.tile([B, 4], f32)
            ssec = sp.tile([B, 4], f32)
            nvt_total = 0
            for s, (soff, swid) in enumerate(zip(SEC_OFF, SECS)):
                nvt = (swid + 511) // 512
                npair = (nvt + 1) // 2
                ps_v = []
                for pp in range(npair):
                    ps_v.append(psB.tile([128, 512], f32, space="PSUM",
                                         tag="mm", name=f"mmps_{s}_{pp}"))
                brow = bp.tile([1, 2048], bf16, tag="brow", name=f"brow_{s}")
                nc.sync.dma_start(brow[:1, :swid],
                                  owT.ap()[K2H:K2H + 1, soff:soff + swid])

                def vregion(vv):
                    pp, vs_ = divmod(vv, 2)
                    w0 = vv * 512
                    w1 = min(swid, w0 + 512)
                    return ps_v[pp][64 * vs_:64 * vs_ + B, :w1 - w0], \
                        (0, 64 * vs_), w0, w1

                # context k-chunks first (ready earlier), then h_new, bias
                korder = list(range(8, 16)) + list(range(8))
                for ki, k in enumerate(korder):
                    wt = wp.tile([128, 2048], bf16, tag="w",
                                 name=f"wt_{s}_{k}")
                    nc.sync.dma_start(wt[:, :swid],
                                      owT.ap()[k * 128:(k + 1) * 128,
                                               soff:soff + swid])
                    for vv in range(nvt):
                        reg, tp, w0, w1 = vregion(vv)
                        nc.tensor.matmul(reg, lhsT=lhsT_for(k),
                                         rhs=wt[:, w0:w1],
                                         start=(ki == 0), stop=False,
                                         tile_position=tp)
                for vv in range(nvt):
                    reg, tp, w0, w1 = vregion(vv)
                    nc.tensor.matmul(reg, lhsT=ones_b[:1, :B],
                                     rhs=brow[:1, w0:w1],
                                     start=False, stop=True,
                                     tile_position=tp)
                sv0 = nvt_total
                for vv in range(nvt):
                    reg, tp, w0, w1 = vregion(vv)
                    nc.vector.tensor_copy(logits[:, soff + w0:soff + w1], reg)
                    nc.vector.tensor_reduce(mx[:, nvt_total:nvt_total + 1],
                                            reg, axis=X, op=ALU.max)
                    nvt_total += 1
                # per-section partial logsumexp, overlapped with next section
                nc.vector.tensor_reduce(msec[:, s:s + 1], mx[:, sv0:nvt_total],
                                        axis=X, op=ALU.max)
                negms = sp.tile([B, 1], f32, tag="negms", name=f"negms_{s}")
                nc.vector.tensor_scalar(negms[:], msec[:, s:s + 1], -1.0,
                                        None, op0=ALU.mult)
                esc = scr.tile([B, 2048], f32, tag="esc", name=f"esc_{s}")
                nc.scalar.activation(esc[:, :swid], logits[:, soff:soff + swid],
                                     AF.Exp, bias=negms[:, :1], scale=1.0,
                                     accum_out=ssec[:, s:s + 1])

            # combine sections: m_c = max_s msec, s_c = sum ssec*exp(msec-m_c)
            m_c = sp.tile([B, 1], f32)
            nc.vector.tensor_reduce(m_c[:], msec[:, :4], axis=X, op=ALU.max)
            d4 = sp.tile([B, 4], f32)
            nc.vector.tensor_scalar(d4[:], msec[:, :4], m_c[:, :1], None,
                                    op0=ALU.subtract)
            nc.scalar.activation(d4[:], d4[:], AF.Exp)
            nc.vector.tensor_tensor(out=d4[:], in0=d4[:], in1=ssec[:, :4],
                                    op=ALU.mult)
            s_c = sp.tile([B, 1], f32)
            nc.vector.tensor_reduce(s_c[:], d4[:, :4], axis=X, op=ALU.add)
            ms = sp.tile([B, 2], f32)
            nc.vector.tensor_copy(ms[:, 0:1], m_c[:])
            nc.vector.tensor_copy(ms[:, 1:2], s_c[:])
            ag4_in = dr.tile([B, 2], f32)
            nc.scalar.dma_start(ag4_in[:], ms[:])
            ag4_out = dr.tile([NCORES * B, 2], f32, addr_space="Shared")
            nc.gpsimd.collective_compute(
                "AllGather", ALU.bypass, replica_groups=rg,
                ins=[ag4_in.opt()], outs=[ag4_out.opt()])
            # land j-major: msall[:, 0:8] = per-rank maxes, [:, 8:16] = sums
            msall = sp.tile([B, 16], f32)
            nc.scalar.dma_start(
                msall[:, :].rearrange("p (j r) -> p j r", r=NCORES),
                ag4_out[:].rearrange("(r p) j -> p j r", p=B))
            mg = sp.tile([B, 1], f32)
            nc.vector.tensor_reduce(mg[:], msall[:, 0:8], axis=X, op=ALU.max)
            d8 = sp.tile([B, 8], f32)
            nc.vector.tensor_scalar(
                d8[:], msall[:, 0:8], mg[:, :1], None, op0=ALU.subtract)
            nc.scalar.activation(d8[:], d8[:], AF.Exp)
            nc.vector.tensor_tensor(
                out=d8[:], in0=d8[:], in1=msall[:, 8:16], op=ALU.mult)
            sg = sp.tile([B, 1], f32)
            nc.vector.tensor_reduce(sg[:], d8[:], axis=X, op=ALU.add)
            nc.scalar.activation(sg[:], sg[:], AF.Ln)
            g_sb = sp.tile([B, 1], f32)
            nc.vector.tensor_tensor(out=g_sb[:], in0=mg[:], in1=sg[:],
                                    op=ALU.add)
            # logp = logits - g, stream out in halves
            for hhalf in range(2):
                c0 = hhalf * (VS // 2)
                c1 = VS if hhalf else VS // 2
                nc.vector.tensor_scalar(logits[:, c0:c1], logits[:, c0:c1],
                                        g_sb[:, :1], None, op0=ALU.subtract)
                nc.scalar.dma_start(logp_o.ap()[:, c0:c1], logits[:, c0:c1])

    nc.compile()
    return nc


def _prep_inputs(inputs):
    last_output = np.asarray(inputs["last_output"]).reshape(B)
    last_hidden = np.asarray(inputs["last_hidden"], dtype=np.float32)
    enc = np.asarray(inputs["encoder_outputs"], dtype=np.float32)
    emb = np.asarray(inputs["embedding"], dtype=np.float32)
    attn_W = np.asarray(inputs["attn_W"], dtype=np.float32)
    v = np.asarray(inputs["v"], dtype=np.float32)
    W_ih = np.asarray(inputs["W_ih"], dtype=np.float32)
    W_hh = np.asarray(inputs["W_hh"], dtype=np.float32)
    b_ih = np.asarray(inputs["b_ih"], dtype=np.float32)
    b_hh = np.asarray(inputs["b_hh"], dtype=np.float32)
    out_W = np.asarray(inputs["out_W"], dtype=np.float32)
    out_b = np.asarray(inputs["out_b"], dtype=np.float32)

    w2 = (attn_W[:, H:].astype(np.float64).T @ v.astype(np.float64))
    w2 = np.ascontiguousarray(
        w2.astype(ml_dtypes.bfloat16).reshape(8, 128).T)   # [128, 8]
    lhT = np.ascontiguousarray(last_hidden.T)              # [1024, 64]
    lhTs = np.ascontiguousarray(
        lhT.reshape(8, 128, B).transpose(1, 0, 2).reshape(128, 8 * B).astype(
            ml_dtypes.bfloat16))

    in_maps = []
    for c in range(NCORES):
        bc = slice(BS * c, BS * (c + 1))
        encT_c = np.ascontiguousarray(
            enc[:, bc, :].transpose(2, 1, 0)).reshape(H, BS * T).astype(
                ml_dtypes.bfloat16)
        enc2_c = np.ascontiguousarray(enc[:, bc, :]).reshape(
            T, BS * H).astype(ml_dtypes.bfloat16)
        rows = np.r_[HS * c:HS * (c + 1),
                     H + HS * c:H + HS * (c + 1),
                     2 * H + HS * c:2 * H + HS * (c + 1)]
        wihT_c = W_ih[rows, :].T                           # [1536, 384]
        wihT_c = np.ascontiguousarray(
            wihT_c.reshape(12, 128, 3 * HS).transpose(1, 0, 2).reshape(
                128, 12 * 3 * HS).astype(ml_dtypes.bfloat16))
        whhT_c = W_hh[rows, :].T                           # [1024, 384]
        whhT_c = np.ascontiguousarray(
            whhT_c.reshape(8, 128, 3 * HS).transpose(1, 0, 2).reshape(
                128, 8 * 3 * HS).astype(ml_dtypes.bfloat16))
        br = (b_ih + b_hh)[rows]
        gbias_c = np.stack([-br[:HS], -br[HS:2 * HS],
                            -2.0 * b_ih[rows][2 * HS:],
                            b_hh[rows][2 * HS:]], axis=1)
        gbias_c = np.ascontiguousarray(gbias_c.astype(np.float32))
        loc = last_output.astype(np.int64) - VS * c
        loc = np.where((loc >= 0) & (loc < VS), loc, VS).astype(np.int32)
        vs = slice(VS * c, VS * (c + 1))
        owT_c = np.ascontiguousarray(
            np.vstack([out_W[vs].T, out_b[vs][None, :]]).astype(
                ml_dtypes.bfloat16))
        in_maps.append({
            "encT": encT_c,
            "enc2": enc2_c,
            "w2": w2,
            "emb_tab": np.ascontiguousarray(emb[vs]),
            "idx": loc[:, None],
            "lhT": lhTs,
            "lh_sl": np.ascontiguousarray(lhT[HS * c:HS * (c + 1)]),
            "wihT": wihT_c,
            "whhT": whhT_c,
            "gbias": gbias_c,
            "owT": owT_c,
        })
    return in_maps


LAST_EXEC_TIME_NS = None
LAST_PROFILE = None


def kernel(**inputs):
    global LAST_EXEC_TIME_NS, LAST_PROFILE
    if "nc" not in _CACHED:
        _CACHED["nc"] = _build()
    nc = _CACHED["nc"]
    in_maps = _prep_inputs(inputs)
    trace = bool(int(os.environ.get("BASS_KERNEL_TRACE", "0")))
    res = run_bass_kernel_spmd(nc, in_maps, list(range(NCORES)), trace=trace)
    LAST_EXEC_TIME_NS = res.exec_time_ns
    LAST_PROFILE = res.profile_json
    r = res.results
    logp = np.concatenate([r[c]["logp"] for c in range(NCORES)], axis=1)
    h_new = np.ascontiguousarray(r[0]["hnT"].T)
    return logp.astype(np.float32), h_new.astype(np.float32)
